# revision 1
# baseline (speedup 1.0000x reference)
# Lagrangian-NN qddot kernel for TRN2 (8 NeuronCores, data-parallel over batch).
#
# Math: scalar L(q,qdot) = MLP(24->256x4->1, softplus). Per sample:
#   M = d2L/dqdot2 + 0.01 I ; C = d2L/dqdot dq ; qddot = M^-1 (dL/dq - C qdot).
# Batched fwd+bwd gives the gradient; 12 qdot-direction forward-over-reverse
# tangents give H[:,12:] whose symmetry supplies both M and the Coriolis
# contraction.
#
# Performance structure (per core, N=1024 samples):
#  - fp16 everywhere except PSUM accumulation and the H/solve stage; weights
#    are pre-converted on the host. DVE TensorTensor ops on all-SBUF packed
#    fp16 run in the 2x_1p perf mode.
#  - One pinned activation table ({exp,ln,abs,identity,copy}) loaded once;
#    softplus/sigmoid are composed as Z = max(A,0)+ln(exp(-|A|)+1),
#    S = exp(A-Z), with max/adds on DVE.
#  - The tangent pass runs as a software pipeline: each 64-sample block is a
#    ~30-step chain over PE/Act/DVE/Pool; blocks are emitted with a skew of
#    SKEW steps so 3-4 independent chains keep every engine queue fed.
#  - fwd/bwd is split into two sample halves: half 0 runs as the pipeline
#    head, half 1 is injected one step per tick into the tangent pipeline of
#    half 0, hiding most of its latency.
#  - M = 0.01(I + K), K = 100*Hqd with ||K|| <= 0.035, so the 12x12 solve is
#    a 3-term Neumann series, computed per 128-sample group inside the
#    pipeline (split across several steps so the in-order DVE queue never
#    head-of-line blocks on the serial solve chain).
import os
import sys
import numpy as np

for p in ("/opt/trn_rl_repo", "/root/.axon_site/_ro/trn_rl_repo"):
    if p not in sys.path:
        sys.path.insert(0, p)

import concourse.bass as bass
import concourse.mybir as mybir
import concourse.tile as tile
from concourse import bacc
from concourse.bass_utils import run_bass_kernel_spmd

F32 = mybir.dt.float32
F16 = mybir.dt.float16
AF = mybir.ActivationFunctionType
ALU = mybir.AluOpType
AX = mybir.AxisListType

B, ND, H, NC = 8192, 12, 256, 8
N = B // NC          # samples per core
HN = N // 2          # samples per fwd/bwd half
IN = 2 * ND          # 24
T = 64               # samples per tangent block
NT = N // T          # 16 blocks
NBH = NT // 2        # blocks per half
NG = N // 128        # 8 groups of 128 samples
FD = ND * T          # 768 tangent free dim
CH = 512             # psum bank chunk (fp32 cols)
KT = H // 128        # 2 k-tiles per hidden dim
SKEW = 4             # tangent pipeline skew (steps between block starts)

_cache = {}


def build_kernel():
    nc = bacc.Bacc("TRN2", target_bir_lowering=False)
    # WBLK columns: WT1|WT2|WT3|Wn1|Wn2|Wn3 (6*256) | W0n (24) | W0qr (768)
    WCOLS = 6 * H + IN + FD
    dx16 = nc.dram_tensor("x16", (N, IN), F16, kind="ExternalInput")
    dqd = nc.dram_tensor("qd32", (N, ND), F32, kind="ExternalInput")
    dwblk = nc.dram_tensor("wblk", (H, WCOLS), F16, kind="ExternalInput")
    dwt0 = nc.dram_tensor("wt0", (IN, H), F16, kind="ExternalInput")
    dsc = nc.dram_tensor("scal", (H, 5), F32, kind="ExternalInput")
    did16 = nc.dram_tensor("id16", (128, 128), F16, kind="ExternalInput")
    did32 = nc.dram_tensor("id32", (128, 128), F32, kind="ExternalInput")
    dout = nc.dram_tensor("qdd", (N, ND), F32, kind="ExternalOutput")

    with tile.TileContext(nc) as tc:
        with tc.tile_pool(name="wp", bufs=1) as wp, \
             tc.tile_pool(name="ap", bufs=1) as ap, \
             tc.tile_pool(name="sc", bufs=2) as sc, \
             tc.tile_pool(name="tg", bufs=2) as tg, \
             tc.tile_pool(name="hp", bufs=1) as hp, \
             tc.tile_pool(name="psB", bufs=3, space="PSUM") as psB, \
             tc.tile_pool(name="psT", bufs=2, space="PSUM") as psT:

            # ---- pin ONE activation table so the compiler's greedy chooser
            # doesn't thrash Exp<->Ln table loads.
            from concourse.hw_specs import get_activation_tables
            need = {AF.Exp, AF.Ln, AF.Abs, AF.Identity, AF.Copy}
            set_id = next(i for i, (_, fns) in
                          enumerate(get_activation_tables(nc.m.arch).items())
                          if need <= fns)
            nc.scalar.add_instruction(mybir.InstLoadActFuncSet(
                name=nc.get_next_instruction_name(), act_func_set_id=set_id,
                ins=[], outs=[]))

            # ---- weight / const loads (batched DMAs) -----------------------
            id16 = wp.tile([128, 128], F16)
            nc.sync.dma_start(id16[:], did16[:])
            id32 = wp.tile([128, 128], F32)
            nc.sync.dma_start(id32[:], did32[:])
            WT0 = wp.tile([IN, H], F16)
            nc.sync.dma_start(WT0[:], dwt0[:])

            WB = [wp.tile([128, WCOLS], F16, tag=f"wb{ki}", name=f"wb{ki}")
                  for ki in range(KT)]
            for ki in range(KT):
                nc.sync.dma_start(WB[ki][:], dwblk[ki * 128:(ki + 1) * 128, :])
            WT = {l: [WB[ki][:, (l - 1) * H:l * H] for ki in range(KT)]
                  for l in (1, 2, 3)}
            Wn = {l: [WB[ki][:, (l + 2) * H:(l + 3) * H] for ki in range(KT)]
                  for l in (1, 2, 3)}
            W0n = [WB[ki][:, 6 * H:6 * H + IN] for ki in range(KT)]
            W0qr = [WB[ki][:, 6 * H + IN:] for ki in range(KT)]
            SCL = [wp.tile([128, 5], F32, tag=f"sc{ki}", name=f"sc{ki}")
                   for ki in range(KT)]
            for ki in range(KT):
                nc.sync.dma_start(SCL[ki][:], dsc[ki * 128:(ki + 1) * 128, :])
            bs = [[SCL[ki][:, l:l + 1] for ki in range(KT)] for l in range(4)]
            w4t = [SCL[ki][:, 4:5] for ki in range(KT)]

            XS = hp.tile([128, NG, IN], F16)
            qd_all = hp.tile([128, NG, ND], F32)
            nc.sync.dma_start(XS[:],
                              dx16[:].rearrange("(g p) c -> p g c", p=128))
            nc.sync.dma_start(qd_all[:],
                              dqd[:].rearrange("(g p) c -> p g c", p=128))

            # ---- XT = X^T [24, N] fp16 -------------------------------------
            XT = hp.tile([IN, N], F16)
            for g in range(NG):
                pt = psT.tile([IN, 128], F16, tag="pt")
                nc.tensor.transpose(pt[:], XS[:, g, :], id16[:])
                nc.scalar.activation(XT[:, g * 128:(g + 1) * 128], pt[:],
                                     AF.Copy)

            def mm(ps_ap, lhsT_list, rhs_list, Fr):
                nk = len(lhsT_list)
                for c0 in range(0, Fr, CH):
                    ce = min(Fr, c0 + CH)
                    for ki in range(nk):
                        nc.tensor.matmul(ps_ap[:, c0:ce], lhsT_list[ki],
                                         rhs_list[ki][:, c0:ce],
                                         start=(ki == 0), stop=(ki == nk - 1))

            # ---- fwd/bwd as per-half step closures -------------------------
            S, F, E1c, c4c, D4c, Zh, Dh, Ah = {}, {}, {}, {}, {}, {}, {}, {}
            Gcp = hp.tile([IN, N], F32)
            gqT = hp.tile([128, NG, ND], F32)

            def fwd_bwd_steps(h):
                hs = slice(h * HN, (h + 1) * HN)
                steps = []

                def mk_fwd_mm(l, ot):
                    def s():
                        if l == 0:
                            lts = [WT0[:][:, ot * 128:(ot + 1) * 128]]
                            rhs = [XT[:, hs]]
                        else:
                            lts = [WT[l][ki][:, ot * 128:(ot + 1) * 128]
                                   for ki in range(KT)]
                            rhs = [Zh[(l - 1, k, h)][:] for k in range(KT)]
                        A16 = sc.tile([128, HN], F16, tag="A16", bufs=3)
                        ps = psB.tile([128, CH], F32, tag="mm")
                        mm(ps[:], lts, rhs, HN)
                        nc.vector.tensor_scalar_add(A16[:], ps[0:128, 0:HN],
                                                    bs[l][ot][:])
                        Ah[(l, ot, h)] = A16
                    return s

                def mk_fwd_act(l, ot):
                    def s():
                        A16 = Ah[(l, ot, h)]
                        ab = sc.tile([128, HN], F16, tag="t1")
                        nc.scalar.activation(ab[:], A16[:], AF.Abs)
                        ex = sc.tile([128, HN], F16, tag="t2")
                        nc.scalar.activation(ex[:], ab[:], AF.Exp, scale=-1.0)
                        Ln = sc.tile([128, HN], F16, tag="L")
                        nc.scalar.activation(Ln[:], ex[:], AF.Ln, bias=1.0)
                        rl = sc.tile([128, HN], F16, tag="t1")
                        nc.vector.tensor_scalar_max(rl[:], A16[:], 0.0)
                        Z = sc.tile([128, HN], F16, tag="Z", bufs=4)
                        nc.vector.tensor_add(Z[:], rl[:], Ln[:])
                        d = sc.tile([128, HN], F16, tag="t2")
                        nc.vector.tensor_sub(d[:], A16[:], Z[:])
                        St = ap.tile([128, HN], F16, tag=f"S{l}_{ot}_{h}")
                        nc.scalar.activation(St[:], d[:], AF.Exp)
                        S[(l, ot, h)] = St
                        Zh[(l, ot, h)] = Z
                    return s

                def mk_d4(ot):
                    def s():
                        S4 = S[(3, ot, h)]
                        Dt4 = ap.tile([128, HN], F16, tag=f"D4_{ot}_{h}")
                        nc.vector.tensor_scalar_mul(Dt4[:], S4[:], w4t[ot][:])
                        D4c[(ot, h)] = Dt4
                        OmS = sc.tile([128, HN], F16, tag="OmS")
                        nc.vector.tensor_scalar(OmS[:], S4[:], -1.0, 1.0,
                                                ALU.mult, ALU.add)
                        tm = sc.tile([128, HN], F16, tag="t1")
                        nc.vector.tensor_mul(tm[:], OmS[:], S4[:])
                        ct = ap.tile([128, HN], F16, tag=f"c4_{ot}_{h}")
                        nc.vector.tensor_scalar_mul(ct[:], tm[:], w4t[ot][:])
                        c4c[(ot, h)] = ct
                    return s

                def mk_bwd_mm(l, ot):
                    def s():
                        lts = [Wn[l + 1][ki][:, ot * 128:(ot + 1) * 128]
                               for ki in range(KT)]
                        if l == 2:
                            rhs = [D4c[(k, h)][:] for k in range(KT)]
                        else:
                            rhs = [Dh[(l + 1, k, h)][:] for k in range(KT)]
                        Ucp = sc.tile([128, HN], F16, tag="Ucp", bufs=3)
                        ps = psB.tile([128, CH], F32, tag="mm")
                        mm(ps[:], lts, rhs, HN)
                        nc.scalar.activation(Ucp[:], ps[0:128, 0:HN], AF.Copy)
                        Ah[("U", l, ot, h)] = Ucp
                    return s

                def mk_bwd_dve(l, ot):
                    def s():
                        Ucp = Ah[("U", l, ot, h)]
                        Dt = sc.tile([128, HN], F16, tag="Dt", bufs=4)
                        nc.vector.tensor_mul(Dt[:], Ucp[:], S[(l, ot, h)][:])
                        Dh[(l, ot, h)] = Dt
                        if l > 0:
                            Ft = ap.tile([128, HN], F16, tag=f"F{l}_{ot}_{h}")
                            nc.vector.tensor_sub(Ft[:], Ucp[:], Dt[:])
                            F[(l, ot, h)] = Ft
                        else:
                            OmS1 = sc.tile([128, HN], F16, tag="OmS")
                            nc.vector.tensor_scalar(OmS1[:], S[(0, ot, h)][:],
                                                    -1.0, 1.0, ALU.mult, ALU.add)
                            Et = ap.tile([128, HN], F16, tag=f"E1_{ot}_{h}")
                            nc.vector.tensor_mul(Et[:], Dt[:], OmS1[:])
                            E1c[(ot, h)] = Et
                    return s

                def mk_g():
                    def s():
                        psG = psB.tile([128, CH], F32, tag="mm")
                        mm(psG[0:IN, 0:HN], [W0n[ki][:] for ki in range(KT)],
                           [Dh[(0, k, h)][:] for k in range(KT)], HN)
                        nc.scalar.activation(Gcp[:, hs], psG[0:IN, 0:HN], AF.Copy)
                    return s

                def mk_gq():
                    def s():
                        for g in range(4 * h, 4 * h + 4):
                            ptg = psT.tile([128, ND], F32, tag="pt")
                            nc.tensor.transpose(
                                ptg[:], Gcp[0:ND, g * 128:(g + 1) * 128],
                                id32[0:ND, 0:ND])
                            nc.vector.tensor_copy(gqT[:, g, :], ptg[:])
                    return s

                for l in range(4):
                    for ot in range(KT):
                        steps.append(mk_fwd_mm(l, ot))
                        steps.append(mk_fwd_act(l, ot))
                steps.append(mk_d4(0))
                steps.append(mk_d4(1))
                for l in (2, 1, 0):
                    for ot in range(KT):
                        steps.append(mk_bwd_mm(l, ot))
                        steps.append(mk_bwd_dve(l, ot))
                steps.append(mk_g())
                steps.append(mk_gq())
                return steps

            # ---- tangent blocks (software-pipelined) -----------------------
            def flat(ts):
                return [t_[:].rearrange("p d t -> p (d t)") for t_ in ts]

            def make_steps(b, Hc_ref):
                i = b & 1
                off = (b % 2) * T
                g = b // 2
                h = b // NBH
                sl = slice((b % NBH) * T, (b % NBH + 1) * T)
                st = {}

                def cf(dct, l=None):
                    if l is None:
                        return lambda ot: dct[(ot, h)]
                    return lambda ot: dct[(l, ot, h)]

                def bcast(cfn, ot):
                    return cfn(ot)[:, sl].unsqueeze(1).broadcast_to((128, ND, T))

                def w0v(ot):
                    return W0qr[ot][:].rearrange("p (d t) -> p d t", d=ND)

                def tgt(tagbase, **kw):
                    kw.setdefault("bufs", 1)
                    return tg.tile([128, ND, T], F16, tag=f"{tagbase}_{i}",
                                   name=tagbase, **kw)

                def s_zd1():
                    st["Zd1"] = []
                    for ot in range(KT):
                        z = tgt(f"Zd1_{ot}")
                        nc.vector.tensor_mul(z[:], w0v(ot), bcast(cf(S, 0), ot))
                        st["Zd1"].append(z)

                def mk_mm(src_key, lW, dst_key, rows=128):
                    def s_mm():
                        pss = []
                        srcs = ([st[src_key]] if isinstance(src_key, str)
                                else [st[k] for k in src_key])
                        for ot in range(KT if rows == 128 else 1):
                            ps = psB.tile([128, FD], F32, tag="mm", name="ps")
                            if rows == 128:
                                lts = [lW[ki][:, ot * 128:(ot + 1) * 128]
                                       for ki in range(KT)]
                            else:
                                lts = [lW[ki][:] for ki in range(KT)]
                            for si, srct in enumerate(srcs):
                                rh = flat(srct)
                                for c0 in range(0, FD, CH):
                                    ce = min(FD, c0 + CH)
                                    for ki in range(KT):
                                        nc.tensor.matmul(
                                            ps[0:rows, c0:ce], lts[ki],
                                            rh[ki][:, c0:ce],
                                            start=(si == 0 and ki == 0),
                                            stop=(si == len(srcs) - 1
                                                  and ki == KT - 1))
                            pss.append(ps)
                        st[dst_key] = pss
                    return s_mm

                def mk_copy(ps_key, dst_key, dtag):
                    def s_copy():
                        st[dst_key] = []
                        for ot in range(KT):
                            cc = tgt(f"{dtag}_{ot}", bufs=2)
                            nc.scalar.activation(cc[:].rearrange("p d t -> p (d t)"),
                                                 st[ps_key][ot][0:128, 0:FD],
                                                 AF.Copy)
                            st[dst_key].append(cc)
                    return s_copy

                def mk_mul(in_key, cfn, dst_key, dtag=None):
                    def s_mul():
                        st[dst_key] = []
                        for ot in range(KT):
                            z = tgt(f"{dtag or dst_key}_{ot}", bufs=2)
                            nc.vector.tensor_mul(z[:], st[in_key][ot][:],
                                                 bcast(cfn, ot))
                            st[dst_key].append(z)
                    return s_mul

                def mk_umul_direct(ps_key, cfn, dst_key):
                    def s_mul():
                        st[dst_key] = []
                        for ot in range(KT):
                            z = tgt(f"u2_{ot}")
                            psv = st[ps_key][ot][0:128, 0:FD].rearrange(
                                "p (d t) -> p d t", d=ND)
                            nc.vector.tensor_mul(z[:], psv, bcast(cfn, ot))
                            st[dst_key].append(z)
                    return s_mul

                def mk_tmul(zd_key, cfn, dst_key, use_w0=False, eng="pool"):
                    def s_mul():
                        st[dst_key] = []
                        for ot in range(KT):
                            z = tgt(f"t_{ot}", bufs=3)
                            src = w0v(ot) if use_w0 else st[zd_key][ot][:]
                            co = bcast(cfn, ot)
                            if eng == "pool":
                                nc.gpsimd.tensor_tensor(z[:], src, co, ALU.mult)
                            else:
                                nc.vector.tensor_mul(z[:], src, co)
                            st[dst_key].append(z)
                    return s_mul

                def mk_add(u_key, t_key, dst_key, dtag, pool_mask=3):
                    def s_add():
                        st[dst_key] = []
                        for ot in range(KT):
                            dd = tgt(f"{dtag}_{ot}")
                            if (pool_mask >> ot) & 1:
                                nc.gpsimd.tensor_add(
                                    dd[:].rearrange("p d t -> p (d t)"),
                                    st[u_key][ot][:].rearrange("p d t -> p (d t)"),
                                    st[t_key][ot][:].rearrange("p d t -> p (d t)"))
                            else:
                                nc.vector.tensor_add(dd[:], st[u_key][ot][:],
                                                     st[t_key][ot][:])
                            st[dst_key].append(dd)
                    return s_add

                def s_hc():
                    if off == 0:
                        Hc_ref[0] = hp.tile([IN, ND, 128], F16, tag="Hc", bufs=2,
                                            name="Hc")
                    nc.scalar.activation(
                        Hc_ref[0][:, :, off:off + T],
                        st["psH"][0][0:IN, 0:FD].rearrange("p (d t) -> p d t",
                                                           d=ND),
                        AF.Copy)

                def s_hstage1():
                    # H extraction; split from the solve so the serial solve
                    # chain never head-of-line blocks the DVE queue
                    if off != T:
                        return
                    ptH = psT.tile([128, 288], F16, tag="pt", name="ptH")
                    for dcol in range(ND):
                        nc.tensor.transpose(ptH[:, dcol * IN:(dcol + 1) * IN],
                                            Hc_ref[0][:, dcol, :],
                                            id16[0:IN, 0:IN])
                    ptHv = ptH[:, 0:ND * IN].rearrange("p (d k) -> p d k", d=ND)
                    Hqg = hp.tile([128, ND, ND], F32, tag="Hqg", bufs=2)
                    nc.scalar.activation(Hqg[:], ptHv[:, :, 0:ND], AF.Copy)
                    Hmg = hp.tile([128, ND, ND], F32, tag="Hmg", bufs=2)
                    nc.scalar.activation(Hmg[:], ptHv[:, :, ND:IN], AF.Copy,
                                         scale=100.0)
                    st["Hqg"], st["Hmg"] = Hqg, Hmg

                def s_hstage2():
                    if off != T:
                        return
                    qdg = qd_all[:, g, :].unsqueeze(1).broadcast_to((128, ND, ND))
                    prodg = hp.tile([128, ND, ND], F32, tag="pg", bufs=2)
                    nc.vector.tensor_tensor(prodg[:], st["Hqg"][:], qdg, ALU.mult)
                    corg = hp.tile([128, ND], F32, tag="cg", bufs=2)
                    nc.vector.tensor_reduce(corg[:].unsqueeze(2), prodg[:],
                                            op=ALU.add, axis=AX.X)
                    rg = hp.tile([128, ND], F32, tag="rg", bufs=2)
                    nc.vector.scalar_tensor_tensor(rg[:], corg[:], -1.0,
                                                   gqT[:, g, :], ALU.mult,
                                                   ALU.add)
                    st["rg"] = rg
                    st["zg"] = rg

                def mk_hiter(last):
                    def s_it():
                        if off != T:
                            return
                        prg = hp.tile([128, ND, ND], F32, tag="pg", bufs=2)
                        nc.vector.tensor_tensor(
                            prg[:], st["Hmg"][:],
                            st["zg"][:].unsqueeze(1).broadcast_to((128, ND, ND)),
                            ALU.mult)
                        sg = hp.tile([128, ND], F32, tag="sg", bufs=2)
                        nc.vector.tensor_reduce(sg[:].unsqueeze(2), prg[:],
                                                op=ALU.add, axis=AX.X)
                        zg = hp.tile([128, ND], F32, tag="zg", bufs=2)
                        nc.vector.scalar_tensor_tensor(zg[:], sg[:], -1.0,
                                                       st["rg"][:], ALU.mult,
                                                       ALU.add)
                        st["zg"] = zg
                        if last:
                            og = hp.tile([128, ND], F32, tag="og", bufs=2)
                            nc.vector.tensor_scalar_mul(og[:], zg[:], 100.0)
                            nc.sync.dma_start(dout[g * 128:(g + 1) * 128, :],
                                              og[:])
                    return s_it

                return [
                    s_zd1,
                    mk_mm("Zd1", WT[1], "psA"),
                    mk_copy("psA", "c2", "c"),
                    mk_mul("c2", cf(S, 1), "Zd2"),
                    mk_mm("Zd2", WT[2], "psB"),
                    mk_copy("psB", "c3", "c"),
                    mk_mul("c3", cf(S, 2), "Zd3"),
                    mk_mm("Zd3", WT[3], "psC"),
                    mk_copy("psC", "cY", "c"),
                    mk_mul("cY", cf(c4c), "Dd4"),
                    mk_mm("Dd4", Wn[3], "psY3"),
                    mk_copy("psY3", "y3", "y"),
                    mk_mul("y3", cf(S, 2), "u3", dtag="u"),
                    mk_tmul("Zd3", cf(F, 2), "t3", eng="pool"),
                    mk_add("u3", "t3", "Dd3", "DdA"),
                    mk_mm("Dd3", Wn[2], "psY2"),
                    mk_umul_direct("psY2", cf(S, 1), "u2"),
                    mk_tmul("Zd2", cf(F, 1), "t2", eng="pool"),
                    mk_add("u2", "t2", "Dd2", "DdB", pool_mask=0),
                    mk_mm("Dd2", Wn[1], "psY1"),
                    mk_copy("psY1", "y1", "y"),
                    mk_mul("y1", cf(S, 0), "u1", dtag="u"),
                    mk_tmul(None, cf(E1c), "t1", use_w0=True, eng="vec"),
                    mk_mm(("u1", "t1"), W0n, "psH", rows=IN),
                    s_hc,
                    s_hstage1,
                    s_hstage2,
                    mk_hiter(False),
                    mk_hiter(False),
                    mk_hiter(True),
                ]

            # head: half 0 fwd/bwd runs serially; half 1 is injected into the
            # tangent pipeline one step per tick.
            for s in fwd_bwd_steps(0):
                s()
            inject = fwd_bwd_steps(1)

            Hc_refs = [[None] for _ in range(NT // 2)]
            steps_all = [make_steps(b, Hc_refs[b // 2]) for b in range(NT)]
            nsteps = len(steps_all[0])
            for tick in range(nsteps + SKEW * (NT - 1) + 1):
                if inject:
                    inject.pop(0)()
                for b in range(NT):
                    j = tick - SKEW * b
                    if 0 <= j < nsteps:
                        steps_all[b][j]()

    nc.compile()
    return nc


def kernel(**inputs):
    f16 = np.float16
    f32 = np.float32
    q = np.asarray(inputs["q"], f32)
    qdot = np.asarray(inputs["qdot"], f32)
    if "nc" not in _cache:
        _cache["nc"] = build_kernel()
    nc = _cache["nc"]
    W = [np.asarray(inputs[f"W{i}"], f32) for i in range(5)]
    X16 = np.ascontiguousarray(np.concatenate([q, qdot], axis=1)).astype(f16)
    wblk = np.concatenate(
        [W[1].T, W[2].T, W[3].T, W[1], W[2], W[3], W[0],
         np.repeat(W[0][:, ND:], T, axis=1)], axis=1).astype(f16)
    scal = np.stack([inputs["b0"], inputs["b1"], inputs["b2"], inputs["b3"],
                     W[4].reshape(H)], axis=1).astype(f32)
    base = {
        "wblk": np.ascontiguousarray(wblk),
        "wt0": np.ascontiguousarray(W[0].T).astype(f16),
        "scal": np.ascontiguousarray(scal),
        "id16": np.eye(128, dtype=f16),
        "id32": np.eye(128, dtype=f32),
    }
    in_maps = []
    for c in range(NC):
        m = dict(base)
        m["x16"] = X16[c * N:(c + 1) * N]
        m["qd32"] = np.ascontiguousarray(qdot[c * N:(c + 1) * N])
        in_maps.append(m)
    res = run_bass_kernel_spmd(nc, in_maps, core_ids=list(range(NC)),
                               trace=bool(os.environ.get("LNN_TRACE")))
    _cache["last"] = res
    out = np.concatenate([res.results[c]["qdd"] for c in range(NC)], axis=0)
    return out.astype(f32)

